# revision 7
# baseline (speedup 1.0000x reference)
"""Trainium2 Bass kernel for KnowledgeGuidedTextGenerator (topk_masking).

Vocab-sharded tensor-parallel across 8 NeuronCores:
  - entity_embeddings.T, logits, relations[:, :, shard] sharded on the
    vocab/entity axis; hidden/W/b replicated.
  - each core: project hidden -> entity space (matmul), sims against its
    entity shard (matmul), local top-1 (max_with_indices), AllGather of
    (val, global_idx) pairs, replicated global-argmax reduce, dma_gather of
    the 4 relation rows for its shard, threshold+OR -> mask, AllGather of
    local "has any valid" bits for the empty-row fallback, masked logits
    via memset(-inf) + copy_predicated.

Self-contained: builds/compiles the Bass program on first call and runs it
via run_bass_kernel_spmd on cores 0-7 with full (host-sharded) inputs.
"""
import numpy as np

import concourse.bass as bass
import concourse.bacc as bacc
import concourse.mybir as mybir
import concourse.tile as tile
from concourse import bass_utils

B = 64          # batch
V = 8192        # vocab / n_entities
D = 256         # d_model
E = 128         # e_dim
R = 4           # n_relations
NCORES = 8
S = V // NCORES  # 1024 vocab shard per core

F32 = mybir.dt.float32
BIG = 65536.0   # > V, exactly representable; idx +- BIG stays exact in f32


def _body(tc, ins, out_d):
    nc = tc.nc
    hT_d, W_d, b_d, ET_d, lg_d, rel_d, aux_d = ins

    with (
        tc.tile_pool(name="main", bufs=1) as pool,
        tc.tile_pool(name="ps", bufs=1, space="PSUM") as psum,
        tc.tile_pool(name="dr", bufs=1, space="DRAM") as dram,
    ):
        # ---- loads ----
        hT_sb = pool.tile([128, 2, B], F32)
        nc.sync.dma_start(hT_sb[:], hT_d.rearrange("(k p) n -> p k n", p=128))
        W_sb = pool.tile([128, 2, E], F32)
        nc.sync.dma_start(W_sb[:], W_d.rearrange("(k p) n -> p k n", p=128))
        b_sb = pool.tile([E, 1], F32)
        nc.sync.dma_start(b_sb[:], b_d[:])
        ET_sb = pool.tile([E, S], F32)
        nc.sync.dma_start(ET_sb[:], ET_d[:])
        lg_sb = pool.tile([B, S], F32)
        nc.sync.dma_start(lg_sb[:], lg_d[:])
        aux_sb = pool.tile([B, 1], F32)
        nc.sync.dma_start(aux_sb[:], aux_d[:])

        # ---- entity projection: eT = W.T @ hidden.T + b  [E, B] ----
        pe = psum.tile([E, B], F32)
        nc.tensor.matmul(pe[:], lhsT=W_sb[:, 0, :], rhs=hT_sb[:, 0, :],
                         start=True, stop=False)
        nc.tensor.matmul(pe[:], lhsT=W_sb[:, 1, :], rhs=hT_sb[:, 1, :],
                         start=False, stop=True)
        eT_sb = pool.tile([E, B], F32)
        nc.scalar.activation(eT_sb[:], pe[:],
                             mybir.ActivationFunctionType.Identity,
                             bias=b_sb[:], scale=1.0)

        # ---- sims = entity_emb @ E_shard.T  [B, S] ----
        ps_s = psum.tile([B, S], F32)
        for j in range(0, S, 512):
            je = min(j + 512, S)
            nc.tensor.matmul(ps_s[:, j:je], lhsT=eT_sb[:], rhs=ET_sb[:, j:je])
        sims_sb = pool.tile([B, S], F32)
        nc.vector.tensor_copy(sims_sb[:], ps_s[:])

        # ---- local top-1 ----
        mx8 = pool.tile([B, 8], F32)
        ix8 = pool.tile([B, 8], mybir.dt.uint32)
        nc.vector.max_with_indices(mx8[:], ix8[:], sims_sb[:])
        gixf = pool.tile([B, 1], F32)
        nc.vector.tensor_copy(gixf[:], ix8[:, 0:1])          # u32 -> f32
        nc.vector.tensor_scalar_add(gixf[:], gixf[:], aux_sb[:])  # + c*S
        pair = pool.tile([B, 2], F32)
        nc.vector.tensor_copy(pair[:, 0:1], mx8[:, 0:1])
        nc.vector.tensor_copy(pair[:, 1:2], gixf[:])

        # ---- AllGather of (val, idx) pairs ----
        ag1_in = dram.tile([B, 2], F32)
        ag1_out = dram.tile([NCORES * B, 2], F32, addr_space="Shared")
        nc.sync.dma_start(ag1_in[:], pair[:])
        nc.gpsimd.collective_compute(
            "AllGather", mybir.AluOpType.bypass,
            replica_groups=[list(range(NCORES))],
            ins=[ag1_in[:]], outs=[ag1_out[:]],
        )

        # ---- global argmax across cores (batch-on-partition layout) ----
        agview = ag1_out[:].rearrange("(k b) t -> b k t", k=NCORES)
        vals_b = pool.tile([B, NCORES], F32)
        idxs_b = pool.tile([B, NCORES], F32)
        nc.sync.dma_start(vals_b[:], agview[:, :, 0].opt())
        nc.sync.dma_start(idxs_b[:], agview[:, :, 1].opt())
        gmax = pool.tile([B, 1], F32)
        nc.vector.reduce_max(gmax[:], vals_b[:], axis=mybir.AxisListType.X)
        eq = pool.tile([B, NCORES], F32)
        nc.vector.tensor_tensor(eq[:], vals_b[:],
                                gmax[:].broadcast_to([B, NCORES]),
                                op=mybir.AluOpType.is_equal)
        candf = pool.tile([B, NCORES], F32)
        nc.vector.scalar_tensor_tensor(candf[:], eq[:], -BIG, idxs_b[:],
                                       op0=mybir.AluOpType.mult,
                                       op1=mybir.AluOpType.add)
        gidxf = pool.tile([B, 1], F32)
        nc.vector.tensor_reduce(gidxf[:], candf[:],
                                axis=mybir.AxisListType.X,
                                op=mybir.AluOpType.min)   # winner_idx - BIG
        idx32 = pool.tile([B, 1], mybir.dt.int32)
        nc.vector.tensor_scalar_add(idx32[:], gidxf[:], BIG)  # f32 -> i32

        # ---- gather relation rows: rel_r[closest[b], :] -> partition b ----
        g_t = []
        for r in range(R):
            gt = pool.tile([B, S], F32, name=f"gt{r}")
            nc.gpsimd.indirect_dma_start(
                out=gt[:], out_offset=None,
                in_=rel_d[r][:],
                in_offset=bass.IndirectOffsetOnAxis(ap=idx32[:, 0:1], axis=0),
            )
            g_t.append(gt)

        # ---- mask: any_r(rel > 0.5) ----
        m01 = pool.tile([B, S], F32)
        nc.vector.tensor_max(m01[:], g_t[0][:], g_t[1][:])
        m23 = pool.tile([B, S], F32)
        nc.vector.tensor_max(m23[:], g_t[2][:], g_t[3][:])
        mall = pool.tile([B, S], F32)
        nc.vector.tensor_max(mall[:], m01[:], m23[:])
        vi8 = pool.tile([B, S], mybir.dt.int8)
        nc.vector.tensor_single_scalar(vi8[:], mall[:], 0.5,
                                       op=mybir.AluOpType.is_gt)

        # ---- empty fallback: AllGather of local max relation values ----
        ha_loc = pool.tile([B, 1], F32)
        nc.vector.reduce_max(ha_loc[:], mall[:], axis=mybir.AxisListType.X)
        ag2_in = dram.tile([B, 1], F32)
        ag2_out = dram.tile([NCORES * B, 1], F32, addr_space="Shared")
        nc.sync.dma_start(ag2_in[:], ha_loc[:])
        nc.gpsimd.collective_compute(
            "AllGather", mybir.AluOpType.bypass,
            replica_groups=[list(range(NCORES))],
            ins=[ag2_in[:]], outs=[ag2_out[:]],
        )
        ha8 = pool.tile([B, NCORES, 1], F32)
        nc.sync.dma_start(ha8[:], ag2_out[:].rearrange("(k b) o -> b k o", k=NCORES))
        g_all = pool.tile([B, 1], F32)
        nc.vector.reduce_max(g_all[:], ha8[:, :, 0], axis=mybir.AxisListType.X)
        empty = pool.tile([B, 1], F32)
        nc.vector.tensor_single_scalar(empty[:], g_all[:], 0.5,
                                       op=mybir.AluOpType.is_le)
        vv = pool.tile([B, S], mybir.dt.int8)
        nc.vector.tensor_scalar_max(vv[:], vi8[:], empty[:])

        # ---- masked logits ----
        outsb = pool.tile([B, S], F32)
        nc.gpsimd.memset(outsb[:], float("-inf"))
        nc.vector.copy_predicated(outsb[:], vv[:], lg_sb[:])
        nc.sync.dma_start(out_d[:], outsb[:])


_CACHE = {}


def _build():
    if "nc" in _CACHE:
        return _CACHE["nc"]
    nc = bacc.Bacc("TRN2", target_bir_lowering=False, debug=False,
                   num_devices=NCORES)
    hT_d = nc.dram_tensor("ht", [D, B], F32, kind="ExternalInput").ap()
    W_d = nc.dram_tensor("wm", [D, E], F32, kind="ExternalInput").ap()
    b_d = nc.dram_tensor("bv", [E, 1], F32, kind="ExternalInput").ap()
    ET_d = nc.dram_tensor("et", [E, S], F32, kind="ExternalInput").ap()
    lg_d = nc.dram_tensor("lg", [B, S], F32, kind="ExternalInput").ap()
    rel_d = [nc.dram_tensor(f"rel{r}", [V, S], F32, kind="ExternalInput").ap()
             for r in range(R)]
    aux_d = nc.dram_tensor("aux", [B, 1], F32, kind="ExternalInput").ap()
    out_d = nc.dram_tensor("out", [B, S], F32, kind="ExternalOutput").ap()

    with tile.TileContext(nc) as tc:
        _body(tc, (hT_d, W_d, b_d, ET_d, lg_d, rel_d, aux_d), out_d)
    nc.compile()
    _CACHE["nc"] = nc
    return nc


def _in_maps(logits, hidden, W_to_entity, b_to_entity, entity_embeddings,
             relations):
    logits = np.asarray(logits, dtype=np.float32)
    hidden = np.asarray(hidden, dtype=np.float32)
    W_to_entity = np.asarray(W_to_entity, dtype=np.float32)
    b_to_entity = np.asarray(b_to_entity, dtype=np.float32)
    entity_embeddings = np.asarray(entity_embeddings, dtype=np.float32)
    relations = np.asarray(relations, dtype=np.float32)

    hT = np.ascontiguousarray(hidden.T)                 # [D, B]
    ET = np.ascontiguousarray(entity_embeddings.T)      # [E, V]
    bv = np.ascontiguousarray(b_to_entity.reshape(E, 1))
    maps = []
    for c in range(NCORES):
        sl = slice(c * S, (c + 1) * S)
        maps.append({
            "ht": hT,
            "wm": W_to_entity,
            "bv": bv,
            "et": ET[:, sl],
            "lg": logits[:, sl],
            "rel0": relations[0, :, sl],
            "rel1": relations[1, :, sl],
            "rel2": relations[2, :, sl],
            "rel3": relations[3, :, sl],
            "aux": np.full((B, 1), c * S, np.float32),
        })
    return maps


def _run(trace=False, **inputs):
    nc = _build()
    maps = _in_maps(**inputs)
    res = bass_utils.run_bass_kernel_spmd(
        nc, maps, core_ids=list(range(NCORES)), trace=trace,
    )
    out = np.concatenate([r["out"] for r in res.results], axis=1)
    return out, res


def kernel(**inputs):
    out, _ = _run(trace=False, **inputs)
    return out
